# revision 1
# baseline (speedup 1.0000x reference)
"""DecoupledIKLoss Trainium2 kernel (8-core data-parallel).

Math: the reference computes ang = atan2(s_raw, c_raw) (degrees) and then
sin/cos of those angles inside DH matrices.  Since
    sin(atan2(s, c)) = s / sqrt(s^2 + c^2),  cos(atan2(s, c)) = c / sqrt(...)
the whole FK chain collapses to a closed form in t = tanh(pred_raw):

    q_i  = a_i^2 + b_i^2           (a = t[:, 0::2], b = t[:, 1::2])
    inv_i = rsqrt(q_i)             (computed as exp(-0.5 * ln(q_i)))
    s_i, c_i = a_i * inv_i, b_i * inv_i
    s23 = s2 c3 + c2 s3 ; c23 = c2 c3 - s2 s3
    u   = 431.8 (c2 + s23) - 20.32 c23
    P5x = c1 u - 139.7 s1
    P5y = s1 u + 139.7 c1
    P5z = 431.8 (c23 - s2) + 20.32 s23 + 671.83

    loss = mean((t - target)^2)
         + 2 * mean(((P5 - P5t)/900)^2)
         + 0.05 * mean((q - 1)^2)

Per-core plan (batch/8 = 524288 rows, tiles of 128 partitions x W rows):
  - inputs cast-DMA'd f32->bf16 (SWDGE) to halve SBUF residency
  - ACT: tanh (to per-joint block layout), ln, exp(-0.5 x)
  - DVE: bf16 2x elementwise FK chain
  - PE:  sum-of-squares reductions: [128,128] "diagonal" matmuls (PSUM
         accumulated) for most terms; the J self term rides a ones-vector
         matmul over the already-computed squares SQ (4x fewer PE ops).
  sc-loss uses sum((t-g)^2) = sum t^2 + sum g^2 - 2 sum t*g, wc-loss uses
  the same expansion; cross terms read the interleaved target tiles through
  strided rhs APs so no layout copies are ever made.
"""

import numpy as np

N_CORES = 8
B = 4194304
BS = B // N_CORES            # rows per core
P = 128                      # SBUF partitions
TILE_WS = [512, 1024, 1024, 1024, 512]   # rows/partition per tile
assert sum(TILE_WS) * P == BS

_L = 431.8                   # DH a2 (== D4 wrist offset)
_A3 = 20.32                  # |DH a3|
_D2 = 139.7                  # DH d2
_D1 = 671.83                 # DH d1
_R = 900.0                   # workspace radius

_BUILT = None
TRACE = False
HOST_SELF_SUMS = False
LAST_EXEC_NS = None
LAST_TRACE_PATH = None


def _build():
    import concourse.tile as tile
    from concourse import bacc, mybir

    f32 = mybir.dt.float32
    bf16 = mybir.dt.bfloat16
    Act = mybir.ActivationFunctionType
    Alu = mybir.AluOpType

    nc = bacc.Bacc("TRN2", target_bir_lowering=False, debug=False,
                   num_devices=N_CORES)

    pred = nc.dram_tensor("pred_raw", [BS, 6], f32, kind="ExternalInput")
    targ = nc.dram_tensor("target_sc", [BS, 6], f32, kind="ExternalInput")
    p5t = nc.dram_tensor("p5_target", [BS, 3], f32, kind="ExternalInput")
    out = nc.dram_tensor("out", [P, 5, 128], f32, kind="ExternalOutput")
    out2 = nc.dram_tensor("out2", [1, 512], f32, kind="ExternalOutput")

    # ones vector for partition-sum matmuls (stationary operand)
    ones_bf = nc.alloc_sbuf_tensor("ones_bf", [P, 1], bf16)
    nc.gpsimd.memset(ones_bf.ap(), 1.0)
    nc.all_engine_barrier()

    with tile.TileContext(nc) as tc:
        with (
            tc.tile_pool(name="inp", bufs=2) as inp,
            tc.tile_pool(name="big", bufs=2) as big,
            tc.tile_pool(name="wk", bufs=1) as wk,
            tc.tile_pool(name="psum", bufs=1, space="PSUM") as psum,
            tc.tile_pool(name="fin", bufs=1) as fin,
        ):
            ps_pos = psum.tile([P, 128], f32)    # sum G*G
            ps_cross = psum.tile([P, 128], f32)  # sum J*G
            ps_wpos = psum.tile([P, 128], f32)   # sum P5*P5 + sum X*X
            ps_wcrs = psum.tile([P, 128], f32)   # sum P5*X
            ps_circ = psum.tile([P, 128], f32)   # sum E*E
            ps_jrow = psum.tile([1, 512], f32)   # ones^T @ SQ  (sum J*J)

            kcs = [w // 128 for w in TILE_WS]
            cnt = {"pos": 0, "cross": 0, "wpos": 0, "wcrs": 0, "circ": 0,
                   "jrow": 0}
            selfmul = 0 if HOST_SELF_SUMS else 1
            tot = {"pos": selfmul * sum(6 * k for k in kcs),
                   "cross": sum(6 * k for k in kcs),
                   "wpos": (3 + 3 * selfmul) * sum(kcs),
                   "wcrs": sum(3 * k for k in kcs),
                   "circ": sum(3 * k for k in kcs),
                   "jrow": sum((6 * w) // 512 for w in TILE_WS)}
            accs = {"pos": ps_pos, "cross": ps_cross, "wpos": ps_wpos,
                    "wcrs": ps_wcrs, "circ": ps_circ, "jrow": ps_jrow}

            def mm(which, lhsT, rhs):
                i = cnt[which]
                cnt[which] += 1
                nc.tensor.matmul(
                    accs[which][:], lhsT, rhs,
                    start=(i == 0), stop=(i == tot[which] - 1),
                    skip_group_check=True,
                )

            def diag_flat(which, a_flat, b_flat, nchunk):
                for k in range(nchunk):
                    sl = slice(k * 128, (k + 1) * 128)
                    mm(which, a_flat[:, sl], b_flat[:, sl])

            row0 = 0
            for t, W in enumerate(TILE_WS):
                KC = W // 128
                rows = P * W
                pred_v = pred.ap()[row0:row0 + rows, :].rearrange(
                    "(p w) c -> p (w c)", p=P)
                targ_v = targ.ap()[row0:row0 + rows, :].rearrange(
                    "(p w) c -> p (w c)", p=P)
                p5t_v = p5t.ap()[row0:row0 + rows, :].rearrange(
                    "(p w) c -> p (w c)", p=P)
                row0 += rows

                # ---- loads (SWDGE cast f32 -> bf16) ----
                pred_sb = inp.tile([P, W, 6], bf16, tag="pred_sb")
                nc.gpsimd.dma_start(out=pred_sb, in_=pred_v)
                tg_sb = inp.tile([P, W, 6], bf16, tag="tg_sb")
                nc.gpsimd.dma_start(out=tg_sb, in_=targ_v)
                p5_sb = inp.tile([P, W, 3], bf16, tag="p5_sb")
                nc.gpsimd.dma_start(out=p5_sb, in_=p5t_v)

                if not HOST_SELF_SUMS:
                    # PE: input self terms (ready as soon as loads land)
                    gf = tg_sb.rearrange("p a b -> p (a b)")
                    diag_flat("pos", gf, gf, 6 * KC)
                    xf = p5_sb.rearrange("p a b -> p (a b)")
                    diag_flat("wpos", xf, xf, 3 * KC)

                # ---- ACT: tanh into block layout [a1|b1|a2|b2|a3|b3] ----
                J = big.tile([P, 6, W], bf16, tag="J")
                nc.scalar.activation(out=J,
                                     in_=pred_sb.transpose([0, 2, 1]),
                                     func=Act.Tanh)
                # sc cross term: block-layout J against strided interleaved G
                for j in range(6):
                    for k in range(KC):
                        sl = slice(k * 128, (k + 1) * 128)
                        mm("cross", J[:, j, sl], tg_sb[:, sl, j])

                # ---- DVE: q = a^2 + b^2 ; circ residual e = q - 1 ----
                SQ = wk.tile([P, 6, W], bf16, tag="SQ")
                nc.vector.tensor_mul(SQ, J, J)
                sqf = SQ.rearrange("p a b -> p (a b)")
                for k in range((6 * W) // 512):
                    mm("jrow", ones_bf.ap(), sqf[:, k * 512:(k + 1) * 512])
                Q = wk.tile([P, 3, W], bf16, tag="Q")
                nc.vector.tensor_add(Q, SQ[:, 0::2, :], SQ[:, 1::2, :])
                E = big.tile([P, 3, W], bf16, tag="E")
                nc.scalar.activation(out=E, in_=Q, func=Act.Copy, bias=-1.0)
                ef = E.rearrange("p a b -> p (a b)")
                diag_flat("circ", ef, ef, 3 * KC)

                # ---- ACT: inv = exp(-0.5 * ln(q)) (rsqrt) ----
                nc.scalar.activation(out=Q, in_=Q, func=Act.Ln)
                INV = wk.tile([P, 3, W], bf16, tag="INV")
                nc.scalar.activation(out=INV, in_=Q, func=Act.Exp,
                                     scale=-0.5)

                # ---- DVE: normalized sin/cos  SC = [s1|c1|s2|c2|s3|c3] ----
                SC = wk.tile([P, 6, W], bf16, tag="SC")
                j4 = J.rearrange("p (j k) w -> p j k w", k=2)
                sc4 = SC.rearrange("p (j k) w -> p j k w", k=2)
                invb = INV.unsqueeze(2).broadcast_to([P, 3, 2, W])
                nc.vector.tensor_mul(sc4, j4, invb)

                s2c2 = SC[:, 2:4, :]
                s3b = SC[:, 4:5, :].broadcast_to([P, 2, W])
                c3b = SC[:, 5:6, :].broadcast_to([P, 2, W])

                MP1 = wk.tile([P, 2, W], bf16, tag="MP1")  # [m1|m3]
                nc.vector.tensor_mul(MP1, s2c2, c3b)
                MP2 = wk.tile([P, 2, W], bf16, tag="MP2")  # [m4|m2]
                nc.vector.tensor_mul(MP2, s2c2, s3b)

                SCC = wk.tile([P, 2, W], bf16, tag="SCC")  # [s23|c23]
                nc.vector.tensor_add(SCC[:, 0, :], MP1[:, 0, :], MP2[:, 1, :])
                nc.vector.tensor_sub(SCC[:, 1, :], MP1[:, 1, :], MP2[:, 0, :])
                s23 = SCC[:, 0, :]
                c23 = SCC[:, 1, :]

                TW = wk.tile([P, 2, W], bf16, tag="TW")   # [t1|w1]
                nc.vector.tensor_add(TW[:, 0, :], SC[:, 3, :], s23)
                nc.vector.tensor_sub(TW[:, 1, :], c23, SC[:, 2, :])
                UW = wk.tile([P, 2, W], bf16, tag="UW")   # [u0|w2]
                nc.vector.tensor_scalar_mul(UW, TW, _L)
                TX = wk.tile([P, 2, W], bf16, tag="TX")   # [t2|w3]
                nc.vector.tensor_scalar_mul(TX[:, 0, :], c23, _A3)
                nc.vector.tensor_scalar(TX[:, 1, :], s23, _A3, _D1,
                                        op0=Alu.mult, op1=Alu.add)

                U = wk.tile([P, W], bf16, tag="U")
                nc.vector.tensor_sub(U, UW[:, 0, :], TX[:, 0, :])

                s1c1 = SC[:, 0:2, :]
                Ub = U.unsqueeze(1).broadcast_to([P, 2, W])
                M57 = wk.tile([P, 2, W], bf16, tag="M57")  # [m7|m5]
                nc.vector.tensor_mul(M57, s1c1, Ub)
                M68 = wk.tile([P, 2, W], bf16, tag="M68")  # [m6|m8]
                nc.vector.tensor_scalar_mul(M68, s1c1, _D2)

                # ---- P5 in block layout [x | y | z] ----
                P5 = big.tile([P, 3, W], bf16, tag="P5")
                nc.vector.tensor_sub(P5[:, 0, :], M57[:, 1, :], M68[:, 0, :])
                nc.vector.tensor_add(P5[:, 1, :], M57[:, 0, :], M68[:, 1, :])
                nc.vector.tensor_add(P5[:, 2, :], UW[:, 1, :], TX[:, 1, :])

                # wc loss on PE: sum(P5^2) + cross vs interleaved target
                pf = P5.rearrange("p a b -> p (a b)")
                diag_flat("wpos", pf, pf, 3 * KC)
                for j in range(3):
                    for k in range(KC):
                        sl = slice(k * 128, (k + 1) * 128)
                        mm("wcrs", P5[:, j, sl], p5_sb[:, sl, j])

            for k, v in cnt.items():
                assert v == tot[k], (k, v, tot[k])

            # ---- epilogue: PSUM -> SBUF -> DRAM ----
            ob = fin.tile([P, 5, 128], f32)
            if HOST_SELF_SUMS:
                nc.vector.memset(ob[:, 0, :], 0.0)
            else:
                nc.vector.tensor_copy(ob[:, 0, :], ps_pos[:])
            nc.vector.tensor_copy(ob[:, 1, :], ps_cross[:])
            nc.vector.tensor_copy(ob[:, 2, :], ps_wpos[:])
            nc.vector.tensor_copy(ob[:, 3, :], ps_wcrs[:])
            nc.vector.tensor_copy(ob[:, 4, :], ps_circ[:])
            nc.sync.dma_start(out.ap(), ob[:])
            ob2 = fin.tile([1, 512], f32)
            nc.vector.tensor_copy(ob2[:], ps_jrow[:])
            nc.sync.dma_start(out2.ap(), ob2[:])

    nc.compile()
    return nc


def _get_built():
    global _BUILT
    if _BUILT is None:
        _BUILT = _build()
    return _BUILT


def kernel(pred_raw, target_sc, P5_target):
    global LAST_EXEC_NS, LAST_TRACE_PATH
    import jax
    # The device run goes through PJRT on the axon platform; a test harness
    # may have pinned jax to cpu for the reference computation.
    if jax.config.jax_platforms != "axon":
        jax.config.update("jax_platforms", "axon")
    from concourse.bass_utils import run_bass_kernel_spmd

    pred_raw = np.ascontiguousarray(pred_raw, dtype=np.float32)
    target_sc = np.ascontiguousarray(target_sc, dtype=np.float32)
    P5_target = np.ascontiguousarray(P5_target, dtype=np.float32)

    nc = _get_built()
    in_maps = []
    for c in range(N_CORES):
        sl = slice(c * BS, (c + 1) * BS)
        in_maps.append({
            "pred_raw": pred_raw[sl],
            "target_sc": target_sc[sl],
            "p5_target": P5_target[sl],
        })
    res = run_bass_kernel_spmd(nc, in_maps, core_ids=list(range(N_CORES)),
                               trace=TRACE)
    LAST_EXEC_NS = res.exec_time_ns
    LAST_TRACE_PATH = (None if res.instructions_and_trace is None
                       else res.instructions_and_trace[1])

    sc = np.float64(0.0)
    wc = np.float64(0.0)
    circ = np.float64(0.0)
    if HOST_SELF_SUMS:
        g = target_sc.astype(np.float64, copy=False)
        x = P5_target.astype(np.float64, copy=False)
        sc += np.einsum("ij,ij->", target_sc, target_sc, dtype=np.float64)
        wc += np.einsum("ij,ij->", P5_target, P5_target, dtype=np.float64)
    for c in range(N_CORES):
        o = res.results[c]["out"].astype(np.float64)   # [P, 5, 128]
        tr = np.einsum('pip->i', o)                    # traces of the accums
        jj = res.results[c]["out2"].astype(np.float64).sum()
        sc += tr[0] + jj - 2.0 * tr[1]
        wc += tr[2] - 2.0 * tr[3]
        circ += tr[4]

    loss = (sc / (6 * B)
            + 2.0 * wc / (_R * _R * 3 * B)
            + 0.05 * circ / (3 * B))
    return np.asarray(np.float32(loss))

